# revision 42
# baseline (speedup 1.0000x reference)
"""Trainium2 Bass kernel for nn_AttentionTF (dense transformer attention block).

Reference computation (per batch b, feature-major x (D, N)):
    q = W_Q x ; k = W_K x ; logits = q^T k  (N, N)
    A = softmax(causal_mask(logits))
    ctx = x A^T ; out = x + W_O^T W_V ctx

Sharding: 8 cores = 4 batches x 2 query-interleavings. Core (b, h) owns the
eight 128-query tiles {2j + h : j = 0..7} of batch b (full 2048 keys,
causality via additive bias + statically truncated key extents). The
interleaving makes the per-slot causal key extent identical across cores, so
one SPMD graph serves all 8 cores while skipping fully-masked key blocks.
No collectives.

Per-core math is re-associated so every matmul is native-layout (the only
transpose is the attention-weight transpose, done on the PE):
    G  = W_Q^T W_K          (d1, d2)   lhsT=W_Q, rhs=W_K
    h  = G^T x_q            (d2, i)    lhsT=G,   rhs=x_q
    S  = h^T x              (i, t)     lhsT=h,   rhs=x       [causal-truncated]
    E  = exp(S + bias - rowmax)        [ACT, accum -> rowsum]
    Et = E^T                (t, i)     PE transpose, 128x128 blocks
    Mt = W_V^T W_O          (e, d)     lhsT=W_V, rhs=W_O
    Yt = x^T Mt             (t, d)     lhsT=x,   rhs=Mt
    pT = Et^T Yt            (i, d)     lhsT=Et,  rhs=Yt      [causal-truncated]
    outT = x_q^T + pT / rowsum         [DVE scalar_tensor_tensor]
Host gathers outT^T into out[b][:, qcols].

All matmul operands are fp16 (f32 PSUM accumulation); measured end-to-end
rel err vs the f32 reference is ~4e-3.
"""

import os
import sys

import numpy as np


def _ensure_import_path():
    try:
        import concourse  # noqa: F401
        return
    except ImportError:
        pass
    for p in ("/opt/trn_rl_repo", "/root/.axon_site/_ro/trn_rl_repo"):
        if os.path.isdir(p) and p not in sys.path:
            sys.path.insert(0, p)
    import concourse  # noqa: F401


_ensure_import_path()

import concourse.bass as bass  # noqa: E402
import concourse.tile as tile  # noqa: E402
from concourse import bacc, mybir  # noqa: E402
from concourse import bass_utils  # noqa: E402
from concourse.masks import make_identity  # noqa: E402

B, D, N, K = 4, 1024, 2048, 1024
NQ = N // 2          # queries per core
NCORES = 8
P = 128              # partitions
DC = D // P          # 8 chunks of the feature dim
TC = N // P          # 16 chunks of the key/seq dim
QC = NQ // P         # 8 query i-tile slots per core
FB = 512             # matmul free-dim block (one PSUM bank of f32)
MASK_VAL = -30000.0  # large-negative causal bias, representable in fp16

# Per-slot causal extents (slot j holds global query tile g = 2j + h).
# Key extent needed: (g+1)*128 keys; max over h in {0,1} gives h-invariant
# static shapes: NT[j] 128-wide key tiles; SCHUNKS[j] = S-chunk widths
# covering 128*NT[j] columns in 512/256-wide pieces.
NT = [2 * j + 2 for j in range(QC)]            # [2,4,6,...,16]


def _chunk_plan(cols):
    plan = []
    while cols > 0:
        w = FB if cols >= FB else cols
        plan.append(w)
        cols -= w
    return plan


SCHUNKS = [_chunk_plan(P * t) for t in NT]
SLOT_ORDER = [7, 6, 5, 0, 4, 1, 3, 2]          # smalls mid-stream

F16 = mybir.dt.float16
F32 = mybir.dt.float32

LAST_EXEC_NS = None
_GRAPH_CACHE = {}


def _build_graph():
    """Build + compile the single-core SPMD Bass graph (same on all 8 cores)."""
    nc = bacc.Bacc("TRN2", target_bir_lowering=False, debug=False,
                   num_devices=NCORES)

    # DRAM I/O. All partition-chunked 3D layouts (128, chunks, free).
    xf_d = nc.dram_tensor("xf", (P, DC, N), F16, kind="ExternalInput")    # x (d,n)
    xq_d = nc.dram_tensor("xq", (P, DC, NQ), F16, kind="ExternalInput")   # x_q (d,i)
    xqt_d = nc.dram_tensor("xqt", (P, QC, D), F16, kind="ExternalInput")  # x_q^T (i,d)
    wq_d = nc.dram_tensor("wq", (P, DC, D), F16, kind="ExternalInput")    # (k,d)
    wk_d = nc.dram_tensor("wk", (P, DC, D), F16, kind="ExternalInput")
    wv_d = nc.dram_tensor("wv", (P, DC, D), F16, kind="ExternalInput")
    wo_d = nc.dram_tensor("wo", (P, DC, D), F16, kind="ExternalInput")
    bias_d = nc.dram_tensor("bias", (P, QC, N), F16, kind="ExternalInput")  # (i,t)
    out_d = nc.dram_tensor("out", (P, QC, D), F16, kind="ExternalOutput")   # (i,d)

    with tile.TileContext(nc) as tc:
        from contextlib import ExitStack
        with ExitStack() as ctx:
            persist = ctx.enter_context(tc.tile_pool(name="persist", bufs=1))
            mm_ps = ctx.enter_context(
                tc.tile_pool(name="mm_ps", bufs=2, space="PSUM"))
            sp_ps = ctx.enter_context(
                tc.tile_pool(name="sp_ps", bufs=4, space="PSUM"))
            tp_ps = ctx.enter_context(
                tc.tile_pool(name="tp_ps", bufs=2, space="PSUM"))

            # Phases 0-2 cycle PSUM groups across mm_ps AND the (then idle)
            # sp_ps pool: 5 accumulation groups in flight instead of 3.
            _ps_state = [0]

            def big_ps():
                pools = (mm_ps, mm_ps, sp_ps, sp_ps, sp_ps, sp_ps)
                pool = pools[_ps_state[0] % 6]
                _ps_state[0] += 1
                return pool.tile([P, FB], F32, tag="ps", name="ps")

            # Persistent input tiles (DMAs issued after the weight chunks so
            # the PE's first G matmuls aren't starved behind these 6 MB).
            xf = persist.tile([P, DC, N], F16)
            xq = persist.tile([P, DC, NQ], F16)

            # Persistent intermediates.
            G = persist.tile([P, DC, D], F16)    # (d1, d2)
            Mt = persist.tile([P, DC, D], F16)   # (e, d)
            h = persist.tile([P, DC, NQ], F16)   # (d2, i)
            Yt = persist.tile([P, TC, D], F16)   # (t, d)

            # ---- Phase 0: weight products G = Wq^T Wk, Mt = Wv^T Wo ----
            # Weights stream per k-chunk so the PE can start ~1.5us in.
            with tc.tile_pool(name="wpool", bufs=1) as wpool:
                wtiles = {}
                for kc in range(DC):
                    for wname, wd in (("wq", wq_d), ("wk", wk_d)):
                        t = wpool.tile([P, D], F16, tag=f"{wname}{kc}",
                                       name=f"{wname}{kc}")
                        if kc < 2:
                            # Early chunks split across queues to land fast.
                            for s in range(4):
                                nc.sync.dma_start(
                                    t[:, 256 * s:256 * (s + 1)],
                                    wd[:, kc, 256 * s:256 * (s + 1)])
                        else:
                            nc.sync.dma_start(t[:], wd[:, kc, :])
                        wtiles[(wname, kc)] = t
                for kc in range(DC):
                    for wname, wd in (("wv", wv_d), ("wo", wo_d)):
                        t = wpool.tile([P, D], F16, tag=f"{wname}{kc}",
                                       name=f"{wname}{kc}")
                        nc.sync.dma_start(t[:], wd[:, kc, :])
                        wtiles[(wname, kc)] = t
                nc.sync.dma_start(xf[:], xf_d[:])
                nc.sync.dma_start(xq[:], xq_d[:])

                for la, lb, dst, ev in (("wq", "wk", G, nc.scalar.copy),
                                        ("wv", "wo", Mt, None)):
                    for j1 in range(DC):          # output row tile
                        for c2 in range(D // FB):  # output column block
                            ps = big_ps()
                            for kc in range(DC):   # contraction over k
                                nc.tensor.matmul(
                                    ps[:],
                                    wtiles[(la, kc)][:, P * j1:P * (j1 + 1)],
                                    wtiles[(lb, kc)][:, FB * c2:FB * (c2 + 1)],
                                    start=(kc == 0), stop=(kc == DC - 1))
                            if ev is not None:
                                ev(dst[:, j1, FB * c2:FB * (c2 + 1)], ps[:])
                            else:
                                nc.vector.tensor_copy(
                                    dst[:, j1, FB * c2:FB * (c2 + 1)], ps[:])

            # Late pools (reuse wpool's address space after it closes).
            late = ctx.enter_context(tc.tile_pool(name="late", bufs=1))
            xqt = late.tile([P, QC, D], F16)
            nc.sync.dma_start(xqt[:], xqt_d[:])
            ident = late.tile([P, P], F16)
            make_identity(nc, ident[:])
            ssb_pool = ctx.enter_context(tc.tile_pool(name="ssb_pool", bufs=2))
            e_pool = ctx.enter_context(tc.tile_pool(name="e_pool", bufs=3))
            et_pool = ctx.enter_context(tc.tile_pool(name="et_pool", bufs=32))
            bias_pool = ctx.enter_context(tc.tile_pool(name="bias_pool", bufs=2))
            out_pool = ctx.enter_context(tc.tile_pool(name="out_pool", bufs=2))
            stat_pool = ctx.enter_context(tc.tile_pool(name="stat_pool", bufs=3))

            # ---- Phase 1: h = G^T x_q  (d2, i) ----
            for j in range(DC):               # output d2-tile
                for ic in range(NQ // FB):
                    ps = big_ps()
                    for j1 in range(DC):      # contraction over d1
                        nc.tensor.matmul(
                            ps[:],
                            G[:, j1, P * j:P * (j + 1)],
                            xq[:, j1, FB * ic:FB * (ic + 1)],
                            start=(j1 == 0), stop=(j1 == DC - 1))
                    nc.scalar.copy(h[:, j, FB * ic:FB * (ic + 1)], ps[:])

            # ---- Phase 2: Yt = x^T Mt  (t, d) ----
            for tt in range(TC):              # output t-tile
                for dc in range(D // FB):
                    # last few groups stay off sp_ps so the attention S
                    # matmuls get their PSUM slots without waiting
                    ps = big_ps() if tt < TC - 2 else mm_ps.tile(
                        [P, FB], F32, tag="ps", name="ps")
                    for ec in range(DC):      # contraction over e
                        nc.tensor.matmul(
                            ps[:],
                            xf[:, ec, P * tt:P * (tt + 1)],
                            Mt[:, ec, FB * dc:FB * (dc + 1)],
                            start=(ec == 0), stop=(ec == DC - 1))
                    if tt % 2 == 0:
                        nc.scalar.copy(Yt[:, tt, FB * dc:FB * (dc + 1)], ps[:])
                    else:
                        nc.vector.tensor_copy(Yt[:, tt, FB * dc:FB * (dc + 1)],
                                              ps[:])

            # ---- Phase 3: per query-slot attention pipeline ----
            softmax_st = {}

            def emit_S(j):
                """S = h_j^T x over the causal key extent; E = exp; rowsum."""
                width = P * NT[j]
                bias_t = bias_pool.tile([P, N], F16, tag="bias",
                                        name=f"bias{j}")
                nc.sync.dma_start(bias_t[:, 0:width], bias_d[:, j, 0:width])
                s_sb = ssb_pool.tile([P, N], F32, tag="ssb", name=f"ssb{j}")
                col = 0
                for w in SCHUNKS[j]:
                    ps = sp_ps.tile([P, FB], F32)
                    for jc in range(DC):      # contraction over d2
                        nc.tensor.matmul(
                            ps[:, 0:w],
                            h[:, jc, P * j:P * (j + 1)],
                            xf[:, jc, col:col + w],
                            start=(jc == 0), stop=(jc == DC - 1))
                    # s_sb = -S + biasneg  (biasneg = 0 valid / +30000 masked)
                    nc.vector.scalar_tensor_tensor(
                        out=s_sb[:, col:col + w],
                        in0=ps[:, 0:w],
                        scalar=-1.0,
                        in1=bias_t[:, col:col + w],
                        op0=mybir.AluOpType.mult,
                        op1=mybir.AluOpType.add)
                    col += w
                mneg = stat_pool.tile([P, 1], F32, tag="mneg", name=f"mneg{j}")
                nc.vector.tensor_reduce(
                    out=mneg[:], in_=s_sb[:, 0:width],
                    axis=mybir.AxisListType.X, op=mybir.AluOpType.min)
                e_t = e_pool.tile([P, N], F16, tag="e", name=f"e{j}")
                rowsum = stat_pool.tile([P, 1], F32, tag="rowsum",
                                        name=f"rowsum{j}")
                # E = exp(-(s_sb) + (-rowmax)) = exp(S - biasneg - rowmax)
                nc.scalar.activation(
                    e_t[:, 0:width], s_sb[:, 0:width],
                    mybir.ActivationFunctionType.Exp,
                    bias=mneg[:], scale=-1.0,
                    accum_out=rowsum[:])
                recip = stat_pool.tile([P, 1], F32, tag="recip",
                                       name=f"recip{j}")
                nc.vector.reciprocal(recip[:], rowsum[:])
                softmax_st[j] = (e_t, recip)

            def emit_TP(j):
                """Transpose E_j, pT = Et^T Yt, outT = xqt + pT/rowsum, DMA."""
                ntj = NT[j]
                e_t, recip = softmax_st.pop(j)
                et_chunks = []
                for c in range(ntj):
                    tps = tp_ps.tile([P, P], F16, tag="tps", name=f"tps{j}_{c}")
                    nc.tensor.transpose(tps[:], e_t[:, P * c:P * (c + 1)],
                                        ident[:])
                    et_sb = et_pool.tile([P, P], F16, tag="et",
                                         name=f"et{j}_{c}")
                    if c % 2 == 0:
                        nc.vector.tensor_copy(et_sb[:], tps[:])
                    else:
                        nc.scalar.copy(et_sb[:], tps[:])
                    et_chunks.append(et_sb)
                out_t = out_pool.tile([P, D], F16, tag="outt", name=f"outt{j}")
                for dc in range(D // FB):
                    ps = mm_ps.tile([P, FB], F32)
                    for c in range(ntj):      # contraction over valid t
                        nc.tensor.matmul(
                            ps[:],
                            et_chunks[c][:],
                            Yt[:, c, FB * dc:FB * (dc + 1)],
                            start=(c == 0), stop=(c == ntj - 1))
                    nc.vector.scalar_tensor_tensor(
                        out=out_t[:, FB * dc:FB * (dc + 1)],
                        in0=ps[:],
                        scalar=recip[:],
                        in1=xqt[:, j, FB * dc:FB * (dc + 1)],
                        op0=mybir.AluOpType.mult,
                        op1=mybir.AluOpType.add)
                nc.sync.dma_start(out_d[:, j, :], out_t[:])

            order = SLOT_ORDER
            emit_S(order[0])
            emit_S(order[1])
            for idx, j in enumerate(order):
                if idx + 2 < len(order):
                    emit_S(order[idx + 2])
                emit_TP(j)

    nc.compile()
    return nc


def _get_graph():
    if "nc" not in _GRAPH_CACHE:
        _GRAPH_CACHE["nc"] = _build_graph()
    return _GRAPH_CACHE["nc"]


def _chunk_p(a, nchunks):
    """(nchunks*128, F) -> (128, nchunks, F) partition-chunked layout."""
    f = a.shape[1]
    return np.ascontiguousarray(a.reshape(nchunks, P, f).swapaxes(0, 1))


def _qidx(hh):
    """Global query indices owned by a core with interleave phase hh."""
    return np.concatenate(
        [np.arange(P * (2 * j + hh), P * (2 * j + hh) + P) for j in range(QC)])


def _host_in_maps(x, W_Q, W_K, W_V, W_O):
    w16 = {name: _chunk_p(np.asarray(w, np.float32).astype(np.float16), DC)
           for name, w in (("wq", W_Q), ("wk", W_K), ("wv", W_V), ("wo", W_O))}

    # Negated causal bias per slot: 0 where key t <= query, +30000 where masked
    tcol = np.arange(N)[None, :]
    bias_h = []
    for hh in range(2):
        gi = _qidx(hh)[:, None]
        bias = np.where(tcol <= gi, np.float16(0.0), np.float16(-MASK_VAL))
        bias_h.append(_chunk_p(bias.astype(np.float16), QC))

    in_maps = []
    for core in range(NCORES):
        b, hh = divmod(core, 2)
        qidx = _qidx(hh)
        xb16 = np.asarray(x[b], np.float32).astype(np.float16)   # (D, N)
        xq16 = np.ascontiguousarray(xb16[:, qidx])               # (D, NQ)
        m = {
            "xf": _chunk_p(xb16, DC),
            "xq": _chunk_p(xq16, DC),
            "xqt": _chunk_p(np.ascontiguousarray(xq16.T), QC),
            "bias": bias_h[hh],
        }
        m.update(w16)
        in_maps.append(m)
    return in_maps


def kernel(inputs, W_Q, W_K, W_V, W_O):
    global LAST_EXEC_NS
    x = np.asarray(inputs, dtype=np.float32)
    nc = _get_graph()
    in_maps = _host_in_maps(x, W_Q, W_K, W_V, W_O)

    trace = os.environ.get("BASS_KERNEL_TRACE", "0") == "1"
    res = bass_utils.run_bass_kernel_spmd(
        nc, in_maps, core_ids=list(range(NCORES)), trace=trace)
    LAST_EXEC_NS = res.exec_time_ns

    out = np.empty_like(x)
    for core in range(NCORES):
        b, hh = divmod(core, 2)
        o = res.results[core]["out"].astype(np.float32)  # (128, QC, D)
        outT = o.swapaxes(0, 1).reshape(NQ, D)  # (i, d) in slot order
        out[b][:, _qidx(hh)] = outT.T
    return out
